# revision 1
# baseline (speedup 1.0000x reference)
"""Gated diagonal linear recurrence (associative-scan gate loop) on 8 TRN2 cores.

Reference computation, per (batch b, channel c):
    inp  = tanh(x[..., :512])
    ig   = sigmoid(x[..., 512:1024]);  og = sigmoid(x[..., 1024:])
    f    = 1 - ig
    h_t  = f_t * h_{t-1} + ig_t * inp_t          (scan over t, h_{-1} = 0)
    y_t  = tanh(h_t) * og_t

Sharding: batch (4) x d_h-half (2) -> 8 cores, no cross-core traffic.
Each core sees three (256, 8192) f32 planes (channel-major so the time
axis lands on the SBUF free dimension) and runs the recurrence with the
DVE tensor_tensor_scan instruction (state = f*state + u along free dim).
"""

import numpy as np

import concourse.bass as bass
import concourse.tile as tile
from concourse import bacc, mybir
from concourse.bass_utils import run_bass_kernel_spmd, checkenv

B, T, DH = 4, 8192, 512
CPT = 128          # channels per partition tile
CPC = 256          # channels per core (= DH / 2)
SEG = 2048         # timesteps per SBUF tile
NSEG = T // SEG
N_CORES = 8

FP32 = mybir.dt.float32

# Filled by kernel() on each run; read by test.py for reporting.
LAST_EXEC_NS = None


def build_nc():
    nc = bacc.Bacc("TRN2", target_bir_lowering=False, debug=False,
                   num_devices=N_CORES)
    xi = nc.declare_dram_parameter("xi", [CPC, T], FP32, isOutput=False)
    gi = nc.declare_dram_parameter("gi", [CPC, T], FP32, isOutput=False)
    go = nc.declare_dram_parameter("go", [CPC, T], FP32, isOutput=False)
    out = nc.declare_dram_parameter("out", [CPC, T], FP32, isOutput=True)

    AF = mybir.ActivationFunctionType
    OP = mybir.AluOpType

    with tile.TileContext(nc) as tc:
        with (
            tc.tile_pool(name="xt", bufs=3) as xt_pool,
            tc.tile_pool(name="gt", bufs=3) as gt_pool,
            tc.tile_pool(name="ot", bufs=3) as ot_pool,
            tc.tile_pool(name="ig", bufs=2) as ig_pool,
            tc.tile_pool(name="h", bufs=4) as h_pool,
            tc.tile_pool(name="th", bufs=2) as th_pool,
        ):
            hprev = [None] * (CPC // CPT)
            for s in range(NSEG):
                cols = slice(s * SEG, (s + 1) * SEG)
                for ct in range(CPC // CPT):
                    rows = slice(ct * CPT, (ct + 1) * CPT)

                    xt = xt_pool.tile([CPT, SEG], FP32)
                    nc.sync.dma_start(xt[:], xi[rows, cols])
                    gt = gt_pool.tile([CPT, SEG], FP32)
                    nc.sync.dma_start(gt[:], gi[rows, cols])
                    ot = ot_pool.tile([CPT, SEG], FP32)
                    nc.sync.dma_start(ot[:], go[rows, cols])

                    ig = ig_pool.tile([CPT, SEG], FP32)
                    nc.scalar.activation(ig[:], gt[:], AF.Sigmoid)
                    # forget gate: 1 - sigmoid(g) == sigmoid(-g), in place
                    nc.scalar.activation(gt[:], gt[:], AF.Sigmoid, scale=-1.0)
                    nc.scalar.activation(xt[:], xt[:], AF.Tanh)
                    # u = tanh(x) * ig, in place into ig
                    nc.vector.tensor_mul(ig[:], xt[:], ig[:])

                    h = h_pool.tile([CPT, SEG], FP32)
                    init = 0.0 if s == 0 else hprev[ct][:, SEG - 1:SEG]
                    nc.vector.tensor_tensor_scan(h[:], gt[:], ig[:], init,
                                                 OP.mult, OP.add)
                    hprev[ct] = h

                    th = th_pool.tile([CPT, SEG], FP32)
                    nc.scalar.activation(th[:], h[:], AF.Tanh)
                    nc.scalar.activation(ot[:], ot[:], AF.Sigmoid)
                    # y = tanh(h) * og, in place into ot
                    nc.vector.tensor_mul(ot[:], th[:], ot[:])
                    nc.sync.dma_start(out[rows, cols], ot[:])

    nc.compile()
    return nc


def shard_inputs(x):
    """Full (B, T, 3*DH) input -> per-core {xi, gi, go}, each (CPC, T)."""
    in_maps = []
    for i in range(N_CORES):
        b, half = divmod(i, 2)
        c0 = half * CPC
        in_maps.append({
            "xi": np.ascontiguousarray(x[b, :, c0:c0 + CPC].T),
            "gi": np.ascontiguousarray(x[b, :, DH + c0:DH + c0 + CPC].T),
            "go": np.ascontiguousarray(x[b, :, 2 * DH + c0:2 * DH + c0 + CPC].T),
        })
    return in_maps


def kernel(x):
    global LAST_EXEC_NS
    x = np.asarray(x, dtype=np.float32)
    assert x.shape == (B, T, 3 * DH), x.shape

    nc = build_nc()
    in_maps = shard_inputs(x)
    res = run_bass_kernel_spmd(nc, in_maps, core_ids=list(range(N_CORES)),
                               trace=bool(checkenv("BASS_TRACE")))
    LAST_EXEC_NS = res.exec_time_ns

    out = np.empty((B, T, DH), dtype=np.float32)
    for i in range(N_CORES):
        b, half = divmod(i, 2)
        c0 = half * CPC
        out[b, :, c0:c0 + CPC] = res.results[i]["out"].T
    return out


# revision 4
# speedup vs baseline: 1.1378x; 1.1378x over previous
"""Gated diagonal linear recurrence (associative-scan gate loop) on 8 TRN2 cores.

Reference computation, per (batch b, channel c):
    inp  = tanh(x[..., :512])
    ig   = sigmoid(x[..., 512:1024]);  og = sigmoid(x[..., 1024:])
    f    = 1 - ig
    h_t  = f_t * h_{t-1} + ig_t * inp_t          (scan over t, h_{-1} = 0)
    y_t  = tanh(h_t) * og_t

Sharding: batch (4) x d_h-half (2) -> 8 cores, no cross-core traffic.
Each core sees three (256, 8192) f32 planes (channel-major so the time
axis lands on the SBUF free dimension) and runs the recurrence with the
DVE tensor_tensor_scan instruction (state = f*state + u along free dim).

Schedule notes:
- in-DMAs on the sync HWDGE ring, out-DMAs on the gpsimd SWDGE ring so a
  not-yet-ready output never head-of-line-blocks input streaming.
- y-mul of iteration k is emitted after scan of iteration k+1 so the DVE
  never stalls waiting for tanh(h_k) on the scalar engine.
- first/last segments are shorter to cut the serial-chain start latency
  and the drain tail.
"""

import numpy as np

import concourse.bass as bass
import concourse.tile as tile
from concourse import bacc, mybir
from concourse.bass_utils import run_bass_kernel_spmd, checkenv

B, T, DH = 4, 8192, 512
CPT = 128          # channels per partition tile
CPC = 256          # channels per core (= DH / 2)
SEGS = [1024, 2048, 2048, 2048, 1024]
assert sum(SEGS) == T
N_CORES = 8

FP32 = mybir.dt.float32
BF16 = mybir.dt.bfloat16

# Filled by kernel() on each run; read by test.py for reporting.
LAST_EXEC_NS = None

import os
USE_BF16_GATES = os.environ.get("KERNEL_BF16", "0") == "1"


def build_nc(bf16_gates=USE_BF16_GATES):
    nc = bacc.Bacc("TRN2", target_bir_lowering=False, debug=False,
                   num_devices=N_CORES)
    xi = nc.declare_dram_parameter("xi", [CPC, T], FP32, isOutput=False)
    gi = nc.declare_dram_parameter("gi", [CPC, T], FP32, isOutput=False)
    go = nc.declare_dram_parameter("go", [CPC, T], FP32, isOutput=False)
    out = nc.declare_dram_parameter("out", [CPC, T], FP32, isOutput=True)

    AF = mybir.ActivationFunctionType
    OP = mybir.AluOpType
    GDT = BF16 if bf16_gates else FP32

    with tile.TileContext(nc) as tc:
        with (
            tc.tile_pool(name="xt", bufs=4) as xt_pool,
            tc.tile_pool(name="gt", bufs=4) as gt_pool,
            tc.tile_pool(name="ot", bufs=4) as ot_pool,
            tc.tile_pool(name="fg", bufs=3) as fg_pool,
            tc.tile_pool(name="ig", bufs=3) as ig_pool,
            tc.tile_pool(name="xg", bufs=3) as xg_pool,
            tc.tile_pool(name="h", bufs=4) as h_pool,
            tc.tile_pool(name="th", bufs=3) as th_pool,
        ):
            nct = CPC // CPT
            hprev = [None] * nct          # (tile, seg_len) per channel tile
            pending = []                  # deferred y-mul + out-DMA work

            def flush_pending():
                th_t, ot_t, rows, cols = pending.pop(0)
                # y = tanh(h) * og, in place into the og tile
                nc.vector.tensor_mul(ot_t[:], th_t[:], ot_t[:])
                nc.gpsimd.dma_start(out[rows, cols], ot_t[:])

            col0 = 0
            for s, seg in enumerate(SEGS):
                cols = slice(col0, col0 + seg)
                for ct in range(nct):
                    rows = slice(ct * CPT, (ct + 1) * CPT)

                    xt = xt_pool.tile([CPT, seg], FP32)
                    nc.sync.dma_start(xt[:], xi[rows, cols])
                    gt = gt_pool.tile([CPT, seg], FP32)
                    nc.sync.dma_start(gt[:], gi[rows, cols])
                    ot = ot_pool.tile([CPT, seg], FP32)
                    nc.sync.dma_start(ot[:], go[rows, cols])

                    ig = ig_pool.tile([CPT, seg], GDT)
                    nc.scalar.activation(ig[:], gt[:], AF.Sigmoid)
                    # forget gate: 1 - sigmoid(g) == sigmoid(-g)
                    if bf16_gates:
                        fg = fg_pool.tile([CPT, seg], GDT)
                        xg = xg_pool.tile([CPT, seg], GDT)
                    else:
                        fg = gt      # in place over gt (dead afterwards)
                        xg = xt      # in place over xt
                    nc.scalar.activation(fg[:], gt[:], AF.Sigmoid, scale=-1.0)
                    nc.scalar.activation(xg[:], xt[:], AF.Tanh)
                    # u = tanh(x) * ig, in place into ig
                    nc.vector.tensor_mul(ig[:], xg[:], ig[:])

                    h = h_pool.tile([CPT, seg], FP32)
                    if s == 0:
                        init = 0.0
                    else:
                        pt, plen = hprev[ct]
                        init = pt[:, plen - 1:plen]
                    nc.vector.tensor_tensor_scan(h[:], fg[:], ig[:], init,
                                                 OP.mult, OP.add)
                    hprev[ct] = (h, seg)

                    th = th_pool.tile([CPT, seg], FP32)
                    nc.scalar.activation(th[:], h[:], AF.Tanh)
                    nc.scalar.activation(ot[:], ot[:], AF.Sigmoid)

                    pending.append((th, ot, rows, cols))
                    # defer y-mul by one iteration: keeps the DVE busy with
                    # u-mul/scan of the next iteration while tanh(h) runs
                    if len(pending) > 1:
                        flush_pending()
                col0 += seg
            while pending:
                flush_pending()

    nc.compile()
    return nc


def shard_inputs(x):
    """Full (B, T, 3*DH) input -> per-core {xi, gi, go}, each (CPC, T)."""
    in_maps = []
    for i in range(N_CORES):
        b, half = divmod(i, 2)
        c0 = half * CPC
        in_maps.append({
            "xi": np.ascontiguousarray(x[b, :, c0:c0 + CPC].T),
            "gi": np.ascontiguousarray(x[b, :, DH + c0:DH + c0 + CPC].T),
            "go": np.ascontiguousarray(x[b, :, 2 * DH + c0:2 * DH + c0 + CPC].T),
        })
    return in_maps


def kernel(x):
    global LAST_EXEC_NS
    x = np.asarray(x, dtype=np.float32)
    assert x.shape == (B, T, 3 * DH), x.shape

    nc = build_nc()
    in_maps = shard_inputs(x)
    res = run_bass_kernel_spmd(nc, in_maps, core_ids=list(range(N_CORES)),
                               trace=bool(checkenv("BASS_TRACE")))
    LAST_EXEC_NS = res.exec_time_ns

    out = np.empty((B, T, DH), dtype=np.float32)
    for i in range(N_CORES):
        b, half = divmod(i, 2)
        c0 = half * CPC
        out[b, :, c0:c0 + CPC] = res.results[i]["out"].T
    return out
